# revision 4
# baseline (speedup 1.0000x reference)
"""Single-head attention kernel for Trainium2, SPMD over 8 NeuronCores.

Problem: x [4,4096,1024] f32 -> q/k/v = x@W+b (head 128) -> softmax(q k^T/sqrt(128)) @ v.
Sharding: core i handles batch i//2, query half i%2. Each core receives its
batch's full x with rows rotated so its 2048 queries are rows 0:2048 (key
order is irrelevant to softmax sums), so all cores run one identical program.
"""

import sys

if "/opt/trn_rl_repo" not in sys.path:
    sys.path.insert(0, "/opt/trn_rl_repo")

import numpy as np

P = 128          # partitions
S = 4096         # sequence length
E = 1024         # n_embd
D = 128          # head size
SQ = 2048        # queries per core
SC = 512         # s-processing chunk
NSC = S // SC    # 8
NEC = E // P     # 8
NKT = S // P     # 32 key tiles
NQC = SQ // SC   # 4 q chunks
SCALE = 1.0 / float(np.sqrt(D))

PV_BF16 = True   # run the P@V stage in bf16 (P in [0,1], V ~ N(0,1); fp32 psum accumulate)

_CACHE = {}


def _build_nc(pv_bf16):
    import concourse.mybir as mybir
    import concourse.tile as tile
    from concourse import bacc

    f32 = mybir.dt.float32
    pv_dt = mybir.dt.bfloat16 if pv_bf16 else f32
    AF = mybir.ActivationFunctionType

    nc = bacc.Bacc(None, target_bir_lowering=False)
    x = nc.dram_tensor("x", [S, E], f32, kind="ExternalInput")
    wq = nc.dram_tensor("wq", [E, D], f32, kind="ExternalInput")
    wk = nc.dram_tensor("wk", [E, D], f32, kind="ExternalInput")
    wv = nc.dram_tensor("wv", [E, D], f32, kind="ExternalInput")
    bq = nc.dram_tensor("bq", [D, 1], f32, kind="ExternalInput")
    bk = nc.dram_tensor("bk", [D, 1], f32, kind="ExternalInput")
    bv = nc.dram_tensor("bv", [D, 1], f32, kind="ExternalInput")
    ident = nc.dram_tensor("ident", [P, P], f32, kind="ExternalInput")
    out = nc.dram_tensor("out", [SQ, D], f32, kind="ExternalOutput")

    with tile.TileContext(nc) as tc:
        with tc.tile_pool(name="const", bufs=1) as constp, \
             tc.tile_pool(name="big", bufs=1) as bigp, \
             tc.tile_pool(name="xp", bufs=8) as xp, \
             tc.tile_pool(name="xtp", bufs=10) as xtp, \
             tc.tile_pool(name="vtmp", bufs=2) as vtmpp, \
             tc.tile_pool(name="pp", bufs=4) as pp, \
             tc.tile_pool(name="op", bufs=4) as op:

            # --- constants in SBUF ---
            wq_sb = constp.tile([P, E], f32)
            wk_sb = constp.tile([P, E], f32)
            wv_sb = constp.tile([P, E], f32)
            for w_dram, w_sb in ((wq, wq_sb), (wk, wk_sb), (wv, wv_sb)):
                for ec in range(NEC):
                    nc.sync.dma_start(out=w_sb[:, ec * P:(ec + 1) * P],
                                      in_=w_dram[ec * P:(ec + 1) * P, :])
            bq_sb = constp.tile([P, 1], f32)
            bk_sb = constp.tile([P, 1], f32)
            bv_sb = constp.tile([P, 1], f32)
            nc.sync.dma_start(out=bq_sb, in_=bq[:, :])
            nc.sync.dma_start(out=bk_sb, in_=bk[:, :])
            nc.sync.dma_start(out=bv_sb, in_=bv[:, :])
            id_sb = constp.tile([P, P], f32)
            nc.sync.dma_start(out=id_sb, in_=ident[:, :])

            # persistent activations
            kT_sb = bigp.tile([P, S], f32)        # K^T  [d, s]
            qT_sb = bigp.tile([P, SQ], f32)       # Q^T  [d, q]
            v_ext = bigp.tile([P, NKT * (D + 1)], pv_dt)  # per key tile: [k,128 V cols + ones col]
            ones_view = v_ext[:, :].rearrange("p (b c) -> p b c", c=D + 1)[:, :, D:D + 1]
            nc.vector.memset(ones_view, 1.0)

            # ---------------- phase 1: transposes + QKV projections ----------------
            with tc.tile_pool(name="tp_ps", bufs=2, space="PSUM") as tp_ps, \
                 tc.tile_pool(name="proj_ps", bufs=1, space="PSUM") as proj_ps, \
                 tc.tile_pool(name="vt_ps", bufs=2, space="PSUM") as vt_ps:
                for sc in range(NSC):
                    xts = []
                    for i in range(4):
                        xt_ = xp.tile([P, E], f32, tag="x")
                        nc.sync.dma_start(
                            out=xt_, in_=x[sc * SC + i * P: sc * SC + (i + 1) * P, :])
                        xts.append(xt_)
                    xTs = []
                    for ec in range(NEC):
                        tp = tp_ps.tile([P, SC], f32, tag="tp")
                        for i in range(4):
                            nc.tensor.matmul(tp[:, i * P:(i + 1) * P],
                                             xts[i][:, ec * P:(ec + 1) * P],
                                             id_sb, start=True, stop=True)
                        xT = xtp.tile([P, SC], f32, tag="xT")
                        nc.vector.tensor_copy(xT, tp)
                        xTs.append(xT)
                    pk = proj_ps.tile([P, SC], f32, tag="pk")
                    pv = proj_ps.tile([P, SC], f32, tag="pv")
                    pq = proj_ps.tile([P, SC], f32, tag="pq", name="pq") if sc < NSC // 2 else None
                    for ec in range(NEC):
                        st, sp_ = (ec == 0), (ec == NEC - 1)
                        nc.tensor.matmul(pk, wk_sb[:, ec * P:(ec + 1) * P], xTs[ec],
                                         start=st, stop=sp_)
                        nc.tensor.matmul(pv, wv_sb[:, ec * P:(ec + 1) * P], xTs[ec],
                                         start=st, stop=sp_)
                        if pq is not None:
                            nc.tensor.matmul(pq, wq_sb[:, ec * P:(ec + 1) * P], xTs[ec],
                                             start=st, stop=sp_)
                    nc.vector.tensor_scalar_add(kT_sb[:, sc * SC:(sc + 1) * SC], pk, bk_sb)
                    if pq is not None:
                        nc.vector.tensor_scalar_add(qT_sb[:, sc * SC:(sc + 1) * SC], pq, bq_sb)
                    # V: add bias, transpose [d,s] -> [s,d], pack into v_ext blocks
                    vtmp = vtmpp.tile([P, SC], f32, tag="vtmp")
                    nc.vector.tensor_scalar_add(vtmp, pv, bv_sb)
                    vt = vt_ps.tile([P, SC], f32, tag="vt")
                    for i in range(4):
                        nc.tensor.matmul(vt[:, i * P:(i + 1) * P],
                                         vtmp[:, i * P:(i + 1) * P],
                                         id_sb, start=True, stop=True)
                    dst = v_ext[:, (sc * 4) * (D + 1):(sc * 4 + 4) * (D + 1)]
                    dst = dst.rearrange("p (b c) -> p b c", c=D + 1)[:, :, 0:D]
                    nc.vector.tensor_copy(dst, vt[:, :].rearrange("p (b c) -> p b c", c=P))

            # ---------------- phase 2: attention ----------------
            with tc.tile_pool(name="sp_ps", bufs=3, space="PSUM") as sp_ps, \
                 tc.tile_pool(name="acc_ps", bufs=4, space="PSUM") as acc_ps:
                for qc in range(NQC):
                    accs = [acc_ps.tile([P, D + 1], f32, tag="acc", name="acc") for _ in range(4)]
                    for kt in range(NKT):
                        sp = sp_ps.tile([P, SC], f32, tag="sp")
                        nc.tensor.matmul(sp, kT_sb[:, kt * P:(kt + 1) * P],
                                         qT_sb[:, qc * SC:(qc + 1) * SC],
                                         start=True, stop=True)
                        p_sb = pp.tile([P, SC], pv_dt, tag="p")
                        nc.scalar.activation(p_sb, sp, AF.Exp, scale=SCALE)
                        for qs in range(4):
                            nc.tensor.matmul(accs[qs],
                                             p_sb[:, qs * P:(qs + 1) * P],
                                             v_ext[:, kt * (D + 1):(kt + 1) * (D + 1)],
                                             start=(kt == 0), stop=(kt == NKT - 1))
                    for qs in range(4):
                        rec = op.tile([P, 1], f32, tag="rec")
                        nc.vector.reciprocal(rec, accs[qs][:, D:D + 1])
                        o_sb = op.tile([P, D], f32, tag="o")
                        nc.vector.tensor_scalar_mul(o_sb, accs[qs][:, 0:D], rec)
                        q0 = (qc * 4 + qs) * P
                        nc.sync.dma_start(out=out[q0:q0 + P, :], in_=o_sb)
    nc.finalize()
    return nc


def _get_nc():
    key = ("nc", PV_BF16)
    if key not in _CACHE:
        _CACHE[key] = _build_nc(PV_BF16)
    return _CACHE[key]


def kernel(x, Wq, bq, Wk, bk, Wv, bv):
    from concourse.bass_utils import run_bass_kernel_spmd

    x = np.asarray(x, dtype=np.float32)
    Wq = np.ascontiguousarray(np.asarray(Wq, dtype=np.float32))
    Wk = np.ascontiguousarray(np.asarray(Wk, dtype=np.float32))
    Wv = np.ascontiguousarray(np.asarray(Wv, dtype=np.float32))
    ident = np.eye(P, dtype=np.float32)
    shared = {
        "wq": Wq, "wk": Wk, "wv": Wv,
        "bq": np.ascontiguousarray(np.asarray(bq, np.float32).reshape(D, 1)),
        "bk": np.ascontiguousarray(np.asarray(bk, np.float32).reshape(D, 1)),
        "bv": np.ascontiguousarray(np.asarray(bv, np.float32).reshape(D, 1)),
        "ident": ident,
    }
    in_maps = []
    for core in range(8):
        b, h = core // 2, core % 2
        xb = x[b] if h == 0 else np.concatenate([x[b, SQ:], x[b, :SQ]], axis=0)
        in_maps.append({"x": np.ascontiguousarray(xb), **shared})

    nc = _get_nc()
    res = run_bass_kernel_spmd(nc, in_maps, core_ids=list(range(8)))

    out = np.empty((4, S, D), dtype=np.float32)
    for core in range(8):
        b, h = core // 2, core % 2
        out[b, h * SQ:(h + 1) * SQ] = res.results[core]["out"]
    return out


# revision 10
# speedup vs baseline: 2.0926x; 2.0926x over previous
"""Single-head attention kernel for Trainium2, SPMD over 8 NeuronCores.

Problem: x [4,4096,1024] f32 -> q/k/v = x@W+b (head 128) -> softmax(q k^T/sqrt(128)) @ v.
Sharding: core i handles batch i//2, query half i%2. Each core receives its
batch's full x with rows rotated so its 2048 queries are rows 0:2048 (key
order is irrelevant to softmax sums), so all cores run one identical program.

Perf notes (from NTFF traces on this hardware):
- fp32 matmul runs in LOW_HIGH 2-pass mode = 4 cycles/row; fp16 is 1 cyc/row
  with an 11-bit mantissa. All values here are O(10), so the whole compute
  path runs in fp16 with fp32 PSUM accumulation (measured ~4e-4 end-to-end).
- DMA-xbar transposes interleaved with regular DMAs thrash xbar_mode and
  serialize the DMA system; transposes run on the PE in transpose-mode
  (1 cyc/row for fp16) instead.
- PSUM accumulation groups: start=True clears the WHOLE bank, so each of the
  8 P@V accumulators gets its own bank-group; P is materialized in SBUF per
  query block and consumed qs-outer so only 4 accumulator banks are live.
- exp on ScalarE costs ~(N+352)/1.2ns per instruction; issued on [128,1024]
  PSUM spans to amortize. x f32->f16 downcasts also run on ScalarE (idle in
  phase 1); PSUM->SBUF copies run on VectorE.
- P@V appends a ones-column to V so the softmax denominator lands in PSUM
  column 128 of each accumulator for free.
"""

import sys

if "/opt/trn_rl_repo" not in sys.path:
    sys.path.insert(0, "/opt/trn_rl_repo")

import numpy as np

P = 128          # partitions
S = 4096         # sequence length
E = 1024         # n_embd
D = 128          # head size
SQ = 2048        # queries per core
SC = 512         # s-processing chunk (phase 1)
NSC = S // SC    # 8
NEC = E // P     # 8
NKT = S // P     # 32 key tiles
QBLK = 1024      # phase-2 query block (ACT instruction width)
NQB = SQ // QBLK # 2
SCALE = 1.0 / float(np.sqrt(D))

_CACHE = {}


def _build_nc():
    import concourse.mybir as mybir
    import concourse.tile as tile
    from concourse import bacc

    f32 = mybir.dt.float32
    f16 = mybir.dt.float16
    AF = mybir.ActivationFunctionType

    nc = bacc.Bacc(None, target_bir_lowering=False)
    x = nc.dram_tensor("x", [S, E], f32, kind="ExternalInput")
    wq = nc.dram_tensor("wq", [E, D], f32, kind="ExternalInput")
    wk = nc.dram_tensor("wk", [E, D], f32, kind="ExternalInput")
    wv = nc.dram_tensor("wv", [E, D], f32, kind="ExternalInput")
    bq = nc.dram_tensor("bq", [D, 1], f32, kind="ExternalInput")
    bk = nc.dram_tensor("bk", [D, 1], f32, kind="ExternalInput")
    bv = nc.dram_tensor("bv", [D, 1], f32, kind="ExternalInput")
    ident = nc.dram_tensor("ident", [P, P], f32, kind="ExternalInput")
    out = nc.dram_tensor("out", [SQ, D], f32, kind="ExternalOutput")

    with tile.TileContext(nc) as tc:
        with tc.tile_pool(name="const", bufs=1) as constp, \
             tc.tile_pool(name="big", bufs=1) as bigp, \
             tc.tile_pool(name="xp", bufs=6) as xp, \
             tc.tile_pool(name="xfp", bufs=6) as xfp, \
             tc.tile_pool(name="xtp", bufs=10) as xtp, \
             tc.tile_pool(name="vtmp", bufs=2) as vtmpp, \
             tc.tile_pool(name="pp", bufs=34) as pp, \
             tc.tile_pool(name="op", bufs=4) as op:

            # --- constants in SBUF (weights staged f32 -> downcast to f16) ---
            w16 = []
            for nm, w_dram in (("wq", wq), ("wk", wk), ("wv", wv)):
                w_st = constp.tile([P, E], f32, name=f"{nm}_st")
                for ec in range(NEC):
                    nc.sync.dma_start(out=w_st[:, ec * P:(ec + 1) * P],
                                      in_=w_dram[ec * P:(ec + 1) * P, :])
                w_sb = constp.tile([P, E], f16, name=f"{nm}16")
                nc.vector.tensor_copy(w_sb, w_st)
                w16.append(w_sb)
            wq_sb, wk_sb, wv_sb = w16
            bq_sb = constp.tile([P, 1], f32)
            bk_sb = constp.tile([P, 1], f32)
            bv_sb = constp.tile([P, 1], f32)
            nc.sync.dma_start(out=bq_sb, in_=bq[:, :])
            nc.sync.dma_start(out=bk_sb, in_=bk[:, :])
            nc.sync.dma_start(out=bv_sb, in_=bv[:, :])
            id_st = constp.tile([P, P], f32)
            nc.sync.dma_start(out=id_st, in_=ident[:, :])
            id16 = constp.tile([P, P], f16)
            nc.vector.tensor_copy(id16, id_st)

            # persistent activations (all fp16)
            kT_sb = bigp.tile([P, S], f16)        # K^T  [d, s]
            qT_sb = bigp.tile([P, SQ], f16)       # Q^T  [d, q]
            v_all = bigp.tile([P, NKT, D + 1], f16)  # [k_local, kt, 128 V | ones]
            nc.vector.memset(v_all[:, :, D:D + 1], 1.0)

            # ---------------- phase 1: x load/downcast/transpose + QKV ----------------
            with tc.tile_pool(name="tp_ps", bufs=2, space="PSUM") as tp_ps, \
                 tc.tile_pool(name="proj_ps", bufs=1, space="PSUM") as proj_ps, \
                 tc.tile_pool(name="vt_ps", bufs=2, space="PSUM") as vt_ps:
                for sc in range(NSC):
                    x16s = []
                    for i in range(4):
                        x_st = xp.tile([P, E], f32, tag="x", name="x")
                        nc.sync.dma_start(
                            out=x_st, in_=x[sc * SC + i * P: sc * SC + (i + 1) * P, :])
                        x16 = xfp.tile([P, E], f16, tag="x16", name="x16")
                        nc.scalar.copy(x16, x_st)          # downcast on ScalarE
                        x16s.append(x16)
                    xTs = []
                    for ec in range(NEC):
                        tp = tp_ps.tile([P, SC], f16, tag="tp", name="tp")
                        for i in range(4):
                            nc.tensor.transpose(tp[:, i * P:(i + 1) * P],
                                                x16s[i][:, ec * P:(ec + 1) * P],
                                                id16)
                        xT = xtp.tile([P, SC], f16, tag="xT", name="xT")
                        nc.vector.tensor_copy(xT, tp)
                        xTs.append(xT)
                    pk = proj_ps.tile([P, SC], f32, tag="pk", name="pk")
                    pv = proj_ps.tile([P, SC], f32, tag="pv", name="pv")
                    pq = proj_ps.tile([P, SC], f32, tag="pq", name="pq") if sc < NSC // 2 else None
                    for ec in range(NEC):
                        st, sp_ = (ec == 0), (ec == NEC - 1)
                        nc.tensor.matmul(pk, wk_sb[:, ec * P:(ec + 1) * P], xTs[ec],
                                         start=st, stop=sp_)
                        nc.tensor.matmul(pv, wv_sb[:, ec * P:(ec + 1) * P], xTs[ec],
                                         start=st, stop=sp_)
                        if pq is not None:
                            nc.tensor.matmul(pq, wq_sb[:, ec * P:(ec + 1) * P], xTs[ec],
                                             start=st, stop=sp_)
                    nc.vector.tensor_scalar_add(kT_sb[:, sc * SC:(sc + 1) * SC], pk, bk_sb)
                    if pq is not None:
                        nc.vector.tensor_scalar_add(qT_sb[:, sc * SC:(sc + 1) * SC], pq, bq_sb)
                    # V: bias add (f32 psum -> f16), PE transpose, pack into v_all
                    vtmp = vtmpp.tile([P, SC], f16, tag="vtmp", name="vtmp")
                    nc.vector.tensor_scalar_add(vtmp, pv, bv_sb)
                    vt = vt_ps.tile([P, SC], f16, tag="vt", name="vt")
                    for i in range(4):
                        nc.tensor.transpose(vt[:, i * P:(i + 1) * P],
                                            vtmp[:, i * P:(i + 1) * P],
                                            id16)
                    nc.vector.tensor_copy(
                        v_all[:, sc * 4:(sc + 1) * 4, 0:D],
                        vt[:, :].rearrange("p (b c) -> p b c", c=P))

            # ---------------- phase 2: attention ----------------
            # Per query block: S^T -> exp into SBUF P tiles (kt-major), then P@V
            # qs-outer so each accumulator owns a full PSUM bank group.
            with tc.tile_pool(name="sp_ps", bufs=2, space="PSUM") as sp_ps, \
                 tc.tile_pool(name="acc_ps", bufs=4, space="PSUM") as acc_ps:
                for qb in range(NQB):
                    p_tiles = []
                    for kt in range(NKT):
                        sp = sp_ps.tile([P, QBLK], f32, tag="sp", name="sp")
                        for h in range(QBLK // SC):
                            nc.tensor.matmul(sp[:, h * SC:(h + 1) * SC],
                                             kT_sb[:, kt * P:(kt + 1) * P],
                                             qT_sb[:, qb * QBLK + h * SC:
                                                   qb * QBLK + (h + 1) * SC],
                                             start=True, stop=True)
                        p_sb = pp.tile([P, QBLK], f16, tag="p", name="p")
                        nc.scalar.activation(p_sb, sp, AF.Exp, scale=SCALE)
                        p_tiles.append(p_sb)
                    for qs in range(QBLK // P):
                        acc = acc_ps.tile([P, D + 1], f32, tag="acc", name="acc")
                        for kt in range(NKT):
                            nc.tensor.matmul(acc,
                                             p_tiles[kt][:, qs * P:(qs + 1) * P],
                                             v_all[:, kt, :],
                                             start=(kt == 0), stop=(kt == NKT - 1))
                        rec = op.tile([P, 1], f32, tag="rec", name="rec")
                        nc.vector.reciprocal(rec, acc[:, D:D + 1])
                        o_sb = op.tile([P, D], f32, tag="o", name="o")
                        nc.vector.tensor_scalar_mul(o_sb, acc[:, 0:D], rec)
                        q0 = (qb * (QBLK // P) + qs) * P
                        nc.sync.dma_start(out=out[q0:q0 + P, :], in_=o_sb)
    nc.finalize()
    return nc


def _get_nc():
    if "nc" not in _CACHE:
        _CACHE["nc"] = _build_nc()
    return _CACHE["nc"]


def _in_maps(x, Wq, bq, Wk, bk, Wv, bv):
    x = np.asarray(x, dtype=np.float32)
    shared = {
        "wq": np.ascontiguousarray(np.asarray(Wq, np.float32)),
        "wk": np.ascontiguousarray(np.asarray(Wk, np.float32)),
        "wv": np.ascontiguousarray(np.asarray(Wv, np.float32)),
        "bq": np.ascontiguousarray(np.asarray(bq, np.float32).reshape(D, 1)),
        "bk": np.ascontiguousarray(np.asarray(bk, np.float32).reshape(D, 1)),
        "bv": np.ascontiguousarray(np.asarray(bv, np.float32).reshape(D, 1)),
        "ident": np.eye(P, dtype=np.float32),
    }
    maps = []
    for core in range(8):
        b, h = core // 2, core % 2
        xb = x[b] if h == 0 else np.concatenate([x[b, SQ:], x[b, :SQ]], axis=0)
        maps.append({"x": np.ascontiguousarray(xb), **shared})
    return maps


def _assemble(results):
    out = np.empty((4, S, D), dtype=np.float32)
    for core in range(8):
        b, h = core // 2, core % 2
        out[b, h * SQ:(h + 1) * SQ] = results[core]["out"]
    return out


def kernel(x, Wq, bq, Wk, bk, Wv, bv):
    from concourse.bass_utils import run_bass_kernel_spmd

    nc = _get_nc()
    res = run_bass_kernel_spmd(nc, _in_maps(x, Wq, bq, Wk, bk, Wv, bv),
                               core_ids=list(range(8)))
    return _assemble(res.results)
